# revision 16
# baseline (speedup 1.0000x reference)
"""Trainium2 Bass kernel for nn_MessageAttention (GNN message passing).

Strategy (8 NeuronCores, SPMD):
  - Shard NODES across cores (6250 nodes/core). On host, sort edges by
    destination node and bucket them per (core, 128-node chunk); every edge's
    destination then lies in its core's node range, so no cross-core
    reduction is needed (outputs are disjoint node slices).
  - Node phase (on device, feat-major): GDBLinear for the k-table (attention
    keys) and the root term, via PE matmuls (bf16 in, fp32 accum),
    kept SBUF-resident in node-major layout.
  - Edge phase (edge-major): per 128-node chunk, per 128-edge tile:
      S[e,n]   = one-hot(idx_local[e] == n)            (DVE is_equal)
      S_T      = PE transpose(S)
      xi       = S_T.T @ k_chunk                        (PE, bf16; replaces a
                                                         DRAM gather of k rows)
      alpha    = per-head reduce(q * xi)                (DVE)
      w        = exp(alpha * scale)                     (ACT; no max-subtraction
                                                         needed: |alpha| < 6)
      M        = [w*q | w] in bf16
      acc     += S.T @ M                                (PE; segment-sum scatter
                                                         of numerators AND
                                                         softmax denominators)
    Finalize per chunk: out = acc_num / max(acc_den, eps) + root.
"""

import math
from contextlib import ExitStack

import numpy as np
import ml_dtypes

import concourse.bass as bass
import concourse.tile as tile
import concourse.tile_sem_assignment as _tsa
from concourse import mybir
from concourse.bass_utils import run_bass_kernel_spmd

F32 = mybir.dt.float32
BF16 = mybir.dt.bfloat16
AX = mybir.AxisListType
OP = mybir.AluOpType
ACT = mybir.ActivationFunctionType

# Problem constants
N, E, H = 50000, 800000, 4
FS, FV, F = 64, 96, 160        # sca feats, vec feats (32*3), total
FM = F + 2 * H                  # 168: [w*q_sca | w*q_vec | w_s | w_v]
CN = 128                        # nodes per chunk
NCORES = 8
SCALE_S = 1.0 / 4.0             # 1/sqrt(dh_sca)=1/sqrt(16)
SCALE_V = 1.0 / math.sqrt(8.0)  # 1/sqrt(dh_vec)=1/sqrt(8)
PAD_IDX = 1000.0                # local idx for padding slots: matches no node

S_ENGINE = "vector"             # "vector" or "gpsimd" for the S one-hot build


def _build_program(npc: int, tpc: int, num_devices: int, nw: int = 512,
                   split_waits: bool = True):
    """Build the per-core Bass program.

    npc: nodes per core; tpc: edge tiles (of 128) per 128-node chunk.
    """
    nch = math.ceil(npc / CN)
    ntiles = nch * tpc
    epc = ntiles * 128

    # Single SW-DMA completion lane: DMA instructions then need at most
    # two sync waits (one engine sem + one DMA sem); walrus's
    # PSEUDO_DMA_DIRECT2D codegen rejects three or more.
    _old_sems = _tsa.NUM_SWDGE_GLOBAL_SEMS
    _tsa.NUM_SWDGE_GLOBAL_SEMS = 1

    nc = bass.Bass("TRN2", num_devices=num_devices)

    qm = nc.dram_tensor("qm", [epc, F], F32, kind="ExternalInput")
    idxcol = nc.dram_tensor("idxcol", [128, ntiles], F32, kind="ExternalInput")
    xts = nc.dram_tensor("xts", [FS, npc], BF16, kind="ExternalInput")
    xtv = nc.dram_tensor("xtv", [FV, npc], BF16, kind="ExternalInput")
    identb = nc.dram_tensor("identb", [128, 128], BF16, kind="ExternalInput")
    colio = nc.dram_tensor("colio", [128, CN], F32, kind="ExternalInput")
    wts = {}
    for p in ("k", "v"):
        for nm, shp in (
            ("Wv1T", [96, 32]), ("Wv2T", [96, 32]), ("Ws1T", [64, 32]),
            ("Ws2aT", [32, 64]), ("Ws2bT", [32, 64]),
            ("WgT", [64, 96]), ("bg", [96, 1]),
        ):
            dt = F32 if nm == "bg" else BF16
            wts[p + nm] = nc.dram_tensor(p + nm, shp, dt, kind="ExternalInput")
    outd = nc.dram_tensor("outd", [npc, F], F32, kind="ExternalOutput")

    with ExitStack() as ctx:
        tc = ctx.enter_context(tile.TileContext(nc))
        const = ctx.enter_context(tc.tile_pool(name="const", bufs=1))
        ktp = ctx.enter_context(tc.tile_pool(name="ktp", bufs=1))

        identb_sb = const.tile([128, 128], BF16)
        nc.gpsimd.dma_start(identb_sb[:], identb[:])
        colio_sb = const.tile([128, CN], F32)
        nc.gpsimd.dma_start(colio_sb[:], colio[:])
        idxcol_sb = const.tile([128, ntiles], F32)
        nc.gpsimd.dma_start(idxcol_sb[:], idxcol[:])
        wsb = {}
        for k, dr in wts.items():
            t = const.tile(list(dr.shape), dr.dtype, tag=k)
            nc.gpsimd.dma_start(t[:], dr[:])
            wsb[k] = t

        # k-table (bf16) and root term (f32), node-major, SBUF-resident:
        # node n -> partition n % 128, cols (n//128)*F ... +F
        kt_sb = ktp.tile([128, nch * F], BF16)
        root_sb = ktp.tile([128, nch * F], F32)

        xall_s = const.tile([FS, npc], BF16)
        nc.gpsimd.dma_start(xall_s[:], xts[:])
        xall_v = const.tile([FV, npc], BF16)
        nc.gpsimd.dma_start(xall_v[:], xtv[:])

        # Edge-phase SBUF pools are opened BEFORE the node phase so their
        # addresses never alias node-phase tiles (avoids cross-phase DMA
        # waits) and edge-phase q prefetch can overlap the node phase.
        esb = ctx.enter_context(tc.tile_pool(name="esb", bufs=2))
        stbp = ctx.enter_context(tc.tile_pool(name="stbp", bufs=3))

        # ---------------- node phase: GDBLinear on x (feat-major) ----------
        with (
            tc.tile_pool(name="nsb", bufs=2) as nsb,
            tc.tile_pool(name="nps", bufs=1, space="PSUM") as nps,
        ):
            ntile_list = [(j * nw, min(nw, npc - j * nw))
                          for j in range(math.ceil(npc / nw))]
            for p, dest in (("k", kt_sb), ("v", root_sb)):
                for (n0, W) in ntile_list:
                    xs = xall_s[:, n0:n0 + W]
                    xv = xall_v[:, n0:n0 + W]

                    # v_inter[t*32+h, n] = sum_c Wv1[h,c] * xv[t*32+c, n]
                    vip = nps.tile([FV, nw], F32, tag="vip")
                    for t in range(3):
                        nc.tensor.matmul(
                            vip[32 * t:32 * t + 32, :W],
                            lhsT=wsb[p + "Wv1T"][32 * t:32 * t + 32, :],
                            rhs=xv[32 * t:32 * t + 32, :],
                            start=True, stop=True)
                    vi = nsb.tile([FV, nw], BF16, tag="vi")
                    nc.scalar.copy(vi[:, :W], vip[:, :W])

                    # vnorm = sqrt(sum_t v_inter^2)
                    sq = nsb.tile([32, nw], F32, tag="sq")
                    sq2 = nsb.tile([32, nw], F32, tag="sq2")
                    nc.vector.tensor_tensor(
                        out=sq[:, :W], in0=vi[0:32, :W], in1=vi[0:32, :W],
                        op=OP.mult)
                    nc.vector.tensor_tensor(
                        out=sq2[:, :W], in0=vi[32:64, :W], in1=vi[32:64, :W],
                        op=OP.mult)
                    nc.vector.tensor_tensor(
                        out=sq[:, :W], in0=sq[:, :W], in1=sq2[:, :W], op=OP.add)
                    nc.vector.tensor_tensor(
                        out=sq2[:, :W], in0=vi[64:96, :W], in1=vi[64:96, :W],
                        op=OP.mult)
                    nc.vector.tensor_tensor(
                        out=sq[:, :W], in0=sq[:, :W], in1=sq2[:, :W], op=OP.add)
                    vn = nsb.tile([32, nw], BF16, tag="vn")
                    nc.scalar.activation(vn[:, :W], sq[:, :W], ACT.Sqrt)

                    # z = Ws1 @ x_sca
                    zp = nps.tile([32, nw], F32, tag="zp")
                    nc.tensor.matmul(zp[:, :W], lhsT=wsb[p + "Ws1T"][:],
                                     rhs=xs[:, :], start=True, stop=True)
                    zs = nsb.tile([32, nw], BF16, tag="zs")
                    nc.scalar.copy(zs[:, :W], zp[:, :W])

                    # out_sca = Ws2 @ [vnorm; z] as two accumulating matmuls
                    osp = nps.tile([FS, nw], F32, tag="osp")
                    nc.tensor.matmul(osp[:, :W], lhsT=wsb[p + "Ws2aT"][:],
                                     rhs=vn[:, :W], start=True, stop=False)
                    nc.tensor.matmul(osp[:, :W], lhsT=wsb[p + "Ws2bT"][:],
                                     rhs=zs[:, :W], start=False, stop=True)
                    osb = nsb.tile([FS, nw], BF16, tag="osb")
                    nc.scalar.copy(osb[:, :W], osp[:, :W])

                    # gate = sigmoid(Wg @ out_sca + bg)
                    gp = nps.tile([FV, nw], F32, tag="gp")
                    nc.tensor.matmul(gp[:, :W], lhsT=wsb[p + "WgT"][:],
                                     rhs=osb[:, :W], start=True, stop=True)
                    g = nsb.tile([FV, nw], F32, tag="g")
                    nc.scalar.activation(g[:, :W], gp[:, :W], ACT.Sigmoid,
                                         bias=wsb[p + "bg"][:, 0:1])

                    # out_vec[t*32+o, n] = gate[o,n] * sum_h Wv2[o,h] vi[t*32+h, n]
                    ovp = nps.tile([FV, nw], F32, tag="ovp")
                    for t in range(3):
                        nc.tensor.matmul(
                            ovp[32 * t:32 * t + 32, :W],
                            lhsT=wsb[p + "Wv2T"][32 * t:32 * t + 32, :],
                            rhs=vi[32 * t:32 * t + 32, :W],
                            start=True, stop=True)
                    ovs = nsb.tile([FV, nw], BF16, tag="ovs")
                    nc.scalar.copy(ovs[:, :W], ovp[:, :W])
                    ovb = nsb.tile([FV, nw], BF16, tag="ovb")
                    nc.vector.tensor_tensor(out=ovb[:, :W], in0=ovs[:, :W],
                                            in1=g[:, :W], op=OP.mult)

                    # transpose to node-major, reorder vec cols (t*32+o -> o*3+t)
                    for b in range(math.ceil(W / 128)):
                        bs = b * 128
                        w = min(128, W - bs)
                        cidx = (n0 + bs) // CN
                        col0 = cidx * F
                        t1 = nps.tile([128, FS], BF16, tag="t1")
                        nc.tensor.transpose(t1[:w, :], in_=osb[:, bs:bs + w],
                                            identity=identb_sb[0:FS, 0:FS])
                        t2 = nps.tile([128, FV], BF16, tag="t2")
                        nc.tensor.transpose(t2[:w, :], in_=ovb[:, bs:bs + w],
                                            identity=identb_sb[0:FV, 0:FV])
                        nc.scalar.copy(dest[:w, col0:col0 + FS], t1[:w, :])
                        nc.vector.tensor_copy(
                            out=dest[:w, col0 + FS:col0 + F].rearrange(
                                "p (o t) -> p o t", t=3),
                            in_=t2[:w, :].rearrange("p (t o) -> p o t", t=3),
                        )

        # ---------------- edge phase ---------------------------------------
        with (
            tc.tile_pool(name="eps", bufs=2, space="PSUM") as eps,
            tc.tile_pool(name="accp", bufs=2, space="PSUM") as accp,
        ):
            for c in range(nch):
                qch = esb.tile([128, tpc * F], F32, tag="qch")
                nc.gpsimd.dma_start(
                    out=qch[:].rearrange("p (t f) -> p t f", f=F),
                    in_=qm[c * tpc * 128:(c + 1) * tpc * 128, :].rearrange(
                        "(t p) f -> p t f", p=128),
                )
                Sb = esb.tile([128, tpc * CN], BF16, tag="Sb")
                xisb = esb.tile([128, tpc * F], F32, tag="xisb")
                for t in range(tpc):
                    gt = c * tpc + t
                    s_eng = nc.gpsimd if S_ENGINE == "gpsimd" else nc.vector
                    s_eng.tensor_scalar(
                        out=Sb[:, t * CN:(t + 1) * CN], in0=colio_sb[:],
                        scalar1=idxcol_sb[:, gt:gt + 1], scalar2=None,
                        op0=OP.is_equal)
                    trp = eps.tile([128, 128], BF16, tag="trp")
                    nc.tensor.transpose(trp[:], in_=Sb[:, t * CN:(t + 1) * CN],
                                        identity=identb_sb[:])
                    stb = stbp.tile([128, 128], BF16, tag="stb")
                    nc.scalar.copy(stb[:], trp[:])
                    xip = eps.tile([128, F], F32, tag="xip")
                    nc.tensor.matmul(xip[:], lhsT=stb[:],
                                     rhs=kt_sb[:, c * F:(c + 1) * F],
                                     start=True, stop=True)
                    nc.scalar.copy(xisb[:, t * F:(t + 1) * F], xip[:])

                # alpha + softmax weights (no max subtraction; |alpha| < 6)
                prod = esb.tile([128, tpc * F], BF16, tag="prod")
                nc.vector.tensor_tensor(out=prod[:], in0=qch[:], in1=xisb[:],
                                        op=OP.mult)
                pv = prod[:].rearrange("p (t f) -> p t f", f=F)
                als = esb.tile([128, tpc * H], F32, tag="als")
                alv = esb.tile([128, tpc * H], F32, tag="alv")
                nc.vector.reduce_sum(
                    out=als[:].rearrange("p (t h) -> p t h", h=H),
                    in_=pv[:, :, 0:FS].rearrange("p t (h d) -> p t h d", d=16),
                    axis=AX.X)
                nc.vector.reduce_sum(
                    out=alv[:].rearrange("p (t h) -> p t h", h=H),
                    in_=pv[:, :, FS:F].rearrange("p t (h d) -> p t h d", d=24),
                    axis=AX.X)

                Mb = esb.tile([128, tpc * FM], BF16, tag="Mb")
                mv = Mb[:].rearrange("p (t f) -> p t f", f=FM)
                nc.scalar.activation(
                    out=mv[:, :, F:F + H],
                    in_=als[:].rearrange("p (t h) -> p t h", h=H),
                    func=ACT.Exp, scale=SCALE_S)
                nc.scalar.activation(
                    out=mv[:, :, F + H:FM],
                    in_=alv[:].rearrange("p (t h) -> p t h", h=H),
                    func=ACT.Exp, scale=SCALE_V)

                qv = qch[:].rearrange("p (t f) -> p t f", f=F)
                nc.vector.tensor_tensor(
                    out=mv[:, :, 0:FS].rearrange("p t (h d) -> p t h d", d=16),
                    in0=qv[:, :, 0:FS].rearrange("p t (h d) -> p t h d", d=16),
                    in1=mv[:, :, F:F + H].to_broadcast([128, tpc, H, 16]),
                    op=OP.mult)
                nc.vector.tensor_tensor(
                    out=mv[:, :, FS:F].rearrange("p t (h d) -> p t h d", d=24),
                    in0=qv[:, :, FS:F].rearrange("p t (h d) -> p t h d", d=24),
                    in1=mv[:, :, F + H:FM].to_broadcast([128, tpc, H, 24]),
                    op=OP.mult)

                # scatter: acc[n, :] += sum_e S[e,n] * M[e, :]
                acc = accp.tile([128, FM], F32, tag="acc")
                for t in range(tpc):
                    nc.tensor.matmul(
                        acc[:], lhsT=Sb[:, t * CN:(t + 1) * CN],
                        rhs=Mb[:, t * FM:(t + 1) * FM],
                        start=(t == 0), stop=(t == tpc - 1))

                # finalize: out = num / max(den, eps) + root
                w8 = esb.tile([128, 2 * H], F32, tag="w8")
                nc.vector.tensor_scalar(
                    out=w8[:], in0=acc[:, F:FM], scalar1=1e-30, scalar2=None,
                    op0=OP.max)
                r8 = esb.tile([128, 2 * H], F32, tag="r8")
                nc.vector.reciprocal(r8[:], w8[:])
                ot = esb.tile([128, F], F32, tag="ot")
                nc.vector.tensor_tensor(
                    out=ot[:, 0:FS].rearrange("p (h d) -> p h d", d=16),
                    in0=acc[:, 0:FS].rearrange("p (h d) -> p h d", d=16),
                    in1=r8[:, 0:H].to_broadcast([128, H, 16]), op=OP.mult)
                nc.vector.tensor_tensor(
                    out=ot[:, FS:F].rearrange("p (h d) -> p h d", d=24),
                    in0=acc[:, FS:F].rearrange("p (h d) -> p h d", d=24),
                    in1=r8[:, H:2 * H].to_broadcast([128, H, 24]), op=OP.mult)
                nc.vector.tensor_tensor(
                    out=ot[:], in0=ot[:], in1=root_sb[:, c * F:(c + 1) * F],
                    op=OP.add)
                ch_n = min(CN, npc - c * CN)
                nc.gpsimd.dma_start(out=outd[c * CN:c * CN + ch_n, :],
                                  in_=ot[:ch_n, :])
    _tsa.NUM_SWDGE_GLOBAL_SEMS = _old_sems
    if split_waits:
        _split_excess_waits(nc)
    return nc


def _split_excess_waits(nc, cap=1):
    """This toolchain's walrus rejects instructions with more than one
    embedded sync wait ("Too many sync wait commands"). Move excess waits
    onto standalone EventSemaphore instructions inserted immediately before
    the instruction on the same engine stream — the sequencer executes them
    in order, so the happens-before relation is preserved."""
    k = 0
    for f in nc.m.functions:
        for bb in f.blocks:
            out = []
            changed = False
            for inst in bb.instructions:
                si = getattr(inst, "sync_info", None)
                if si is not None and len(si.on_wait) > cap:
                    for w in si.on_wait[cap:]:
                        k += 1
                        ev = mybir.InstEventSemaphore(
                            name=f"WSPLIT-{k}", ins=[], outs=[],
                            sync_info=mybir.SyncInfo(on_wait=[w],
                                                     on_update=[]))
                        ev.engine = inst.engine
                        out.append(ev)
                    inst.sync_info = mybir.SyncInfo(
                        on_wait=list(si.on_wait[:cap]),
                        on_update=list(si.on_update))
                    changed = True
                out.append(inst)
            if changed:
                bb.instructions = out


def _prep_inputs(x_sca, x_vec, q_sca, q_vec, edge_index_i, wparams,
                 ncores, npc):
    """Host-side sharding: sort edges by destination, bucket per
    (core, 128-node chunk), pad chunks to a uniform tile count."""
    nE = q_sca.shape[0]
    idx = np.asarray(edge_index_i).astype(np.int64).ravel()
    order = np.argsort(idx, kind="stable")
    idxs = idx[order]
    nch = math.ceil(npc / CN)

    core = idxs // npc
    local = idxs - core * npc
    chunk = local // CN
    cin = (local - chunk * CN).astype(np.float32)
    gchunk = core * nch + chunk
    counts = np.bincount(gchunk, minlength=ncores * nch)
    tpc = max(1, int(math.ceil(counts.max() / 128.0)))
    ntiles = nch * tpc
    epc = ntiles * 128

    qcat = np.concatenate(
        [np.asarray(q_sca, dtype=np.float32),
         np.asarray(q_vec, dtype=np.float32).reshape(nE, FV)], axis=1)

    # slot within the padded layout: chunk base + rank within chunk
    chunk_starts = np.zeros(ncores * nch + 1, dtype=np.int64)
    np.cumsum(counts, out=chunk_starts[1:])
    rank = np.arange(len(idxs), dtype=np.int64) - chunk_starts[gchunk]
    slot_global = (gchunk % nch) * (tpc * 128) + rank  # slot within the core

    ident_b = np.eye(128, dtype=ml_dtypes.bfloat16)
    colio_np = np.tile(np.arange(CN, dtype=np.float32), (128, 1))

    wts = {}
    for p in ("k", "v"):
        bft = lambda a: np.ascontiguousarray(a).astype(ml_dtypes.bfloat16)
        wts[p + "Wv1T"] = bft(np.tile(np.asarray(wparams[p + "_Wv1"], np.float32).T, (3, 1)))
        wts[p + "Wv2T"] = bft(np.tile(np.asarray(wparams[p + "_Wv2"], np.float32).T, (3, 1)))
        wts[p + "Ws1T"] = bft(wparams[p + "_Ws1"].T)
        ws2t = np.asarray(wparams[p + "_Ws2"], np.float32).T
        wts[p + "Ws2aT"] = bft(ws2t[:32])
        wts[p + "Ws2bT"] = bft(ws2t[32:])
        wts[p + "WgT"] = bft(np.tile(np.asarray(
            wparams[p + "_Wg"], np.float32).T, (1, 3)))
        wts[p + "bg"] = np.ascontiguousarray(np.tile(
            np.asarray(wparams[p + "_bg"], np.float32).reshape(32, 1), (3, 1)))

    x_sca = np.asarray(x_sca, dtype=np.float32)
    x_vec = np.asarray(x_vec, dtype=np.float32)

    in_maps = []
    for c in range(ncores):
        sel = core == c
        qm_c = np.zeros([epc, F], np.float32)
        qm_c[slot_global[sel]] = qcat[order[sel]]
        idxcol_c = np.full([128, ntiles], PAD_IDX, np.float32)
        sl = slot_global[sel]
        idxcol_c[sl % 128, sl // 128] = cin[sel]
        n0 = c * npc
        in_maps.append(dict(
            qm=qm_c,
            idxcol=idxcol_c,
            xts=np.ascontiguousarray(
                x_sca[n0:n0 + npc].T).astype(ml_dtypes.bfloat16),
            xtv=np.ascontiguousarray(
                x_vec[n0:n0 + npc].transpose(2, 1, 0).reshape(
                    FV, npc)).astype(ml_dtypes.bfloat16),
            identb=ident_b,
            colio=colio_np,
            **wts,
        ))
    return in_maps, tpc


_PROG_CACHE = {}


def kernel(x_sca, x_vec, q_sca, q_vec, edge_index_i, **wparams):
    npc = N // NCORES
    in_maps, tpc = _prep_inputs(
        x_sca, x_vec, q_sca, q_vec, edge_index_i, wparams, NCORES, npc)

    key = (npc, tpc, NCORES)
    if key not in _PROG_CACHE:
        _PROG_CACHE[key] = _build_program(npc, tpc, NCORES)
    nc = _PROG_CACHE[key]

    res = run_bass_kernel_spmd(nc, in_maps, list(range(NCORES)))
    out = np.concatenate([res.results[c]["outd"] for c in range(NCORES)],
                         axis=0)
    out_sca = np.ascontiguousarray(out[:, 0:FS])
    out_vec = np.ascontiguousarray(out[:, FS:F]).reshape(N, 32, 3)
    return out_sca, out_vec
